# revision 1
# baseline (speedup 1.0000x reference)
"""GGNN message-passing kernel for Trainium2 (8 NeuronCores, data-parallel).

Reference semantics (B=16, N=1024, D=512, 3 steps):
    a_in  = in_matrix  @ nodes          [B,N,D]
    a_out = out_matrix @ nodes          [B,N,D]
    av    = concat(a_in, a_out)         [B,N,2D]
    u3    = nodes @ w3u.T
    zv    = sigmoid(av @ w3w.T + b3w + u3 + b3u)
    rv    = sigmoid(av @ w4w.T + b4w + u3 + b3u)
    hv    = tanh(av @ w5w.T + b5w + (rv*nodes) @ w5u.T + b5u)
    nodes = (1-zv)*nodes + zv*hv
Returns (nodes, in_matrix).

Layout: per core 2 batches; node state kept feature-major H^T[D,N] in SBUF
as fp32r; adjacency transposes + linear weights pre-transposed on host so
activations are always the moving matmul operand.  One on-device PE
transpose of H per step feeds the graph matmuls.
"""
import json
import numpy as np

import concourse.bass as bass
import concourse.mybir as mybir
from concourse import tile
from concourse.bass_utils import run_bass_kernel_spmd
from concourse.vector_clock import ScopedClock, VectorClock

f32 = mybir.dt.float32
f32r = mybir.dt.float32r
AF = mybir.ActivationFunctionType

B, N, D = 16, 1024, 512
STEPS = 3
NCORES = 8
BL = B // NCORES          # batches per core
DT = D // 128             # 4 feature tiles
NT = N // 128             # 8 node tiles
MC = 2                    # n-chunks of 512 per step phase
CH = N // MC              # 512


class CompatTileContext(tile.TileContext):
    """Tail drain emitted one-wait-per-proc (this walrus encodes a single
    sync wait per instruction)."""

    def _drain_and_barrier(self, tick_clock, wait_clock):
        gc = tick_clock.global_clock
        for p in range(len(gc)):
            t = gc[p]
            if t == 0:
                continue
            vc = VectorClock()
            vc.require_at_least(p, t)
            d = self.nc.sync.drain()
            wait_clock.add_sem_waits(d.ins, ScopedClock({None: vc}))

        self.nc.all_engine_barrier()
        assert self.sems is not None
        popped = self.nc._tile_sem_poison_stack.pop()
        assert popped is self._sem_poison
        self.nc.clear_and_free_semaphores(list(self.sems.allocated().values()))
        self.nc.all_engine_barrier()


def _install_wait_splitter(nc):
    """Hoist all-but-one on_wait of every instruction onto injected
    same-engine NoOps at BIR-serialization time."""
    orig = nc.to_json_bytes

    def patched():
        m = json.loads(orig())
        ctr = 0
        for fn in m.get("functions", []):
            for bb in fn.get("blocks", []):
                new = []
                for inst in bb.get("instructions", []):
                    si = inst.get("sync_info")
                    ow = (si or {}).get("on_wait") or []
                    if len(ow) > 1:
                        for w in ow[:-1]:
                            ctr += 1
                            new.append({
                                "name": f"WSPL-{ctr}",
                                "opcode": "NoOp",
                                "engine": inst["engine"],
                                "ins": [],
                                "outs": [],
                                "debug": inst.get("debug", 0),
                                "sync_info": {"on_wait": [w], "on_update": []},
                            })
                        si["on_wait"] = [ow[-1]]
                    new.append(inst)
                bb["instructions"] = new
        return json.dumps(m).encode()

    nc.to_json_bytes = patched


def build_nc():
    nc = bass.Bass()
    x_d = nc.dram_tensor("x", [BL, D, N], f32, kind="ExternalInput")
    intt_d = nc.dram_tensor("intt", [N, N], f32, kind="ExternalInput")
    outt_d = nc.dram_tensor("outt", [N, N], f32, kind="ExternalInput")
    w3wt_d = nc.dram_tensor("w3wt", [2 * D, D], f32, kind="ExternalInput")
    w4wt_d = nc.dram_tensor("w4wt", [2 * D, D], f32, kind="ExternalInput")
    w5wt_d = nc.dram_tensor("w5wt", [2 * D, D], f32, kind="ExternalInput")
    w3ut_d = nc.dram_tensor("w3ut", [D, D], f32, kind="ExternalInput")
    w5ut_d = nc.dram_tensor("w5ut", [D, D], f32, kind="ExternalInput")
    bz_d = nc.dram_tensor("bz", [D], f32, kind="ExternalInput")
    br_d = nc.dram_tensor("br", [D], f32, kind="ExternalInput")
    bh_d = nc.dram_tensor("bh", [D], f32, kind="ExternalInput")
    id_d = nc.dram_tensor("ident", [128, 128], f32, kind="ExternalInput")
    y_d = nc.dram_tensor("y", [BL, D, N], f32, kind="ExternalOutput")

    with CompatTileContext(nc) as tc:
        with tc.tile_pool(name="wp", bufs=1) as wp, \
             tc.tile_pool(name="hp", bufs=1) as hp, \
             tc.tile_pool(name="hn", bufs=1) as hn, \
             tc.tile_pool(name="avp", bufs=1) as avp, \
             tc.tile_pool(name="rvfp", bufs=1) as rvfp, \
             tc.tile_pool(name="hvp", bufs=1) as hvp, \
             tc.tile_pool(name="sc", bufs=3) as sc, \
             tc.tile_pool(name="ztp", bufs=4) as ztp, \
             tc.tile_pool(name="mm", bufs=5, space="PSUM") as mm, \
             tc.tile_pool(name="tp", bufs=3, space="PSUM") as tp:

            # --- weights: gpsimd casting DMAs fp32 -> fp32r, chunked per n-tile
            intt_sb = wp.tile([128, NT, N], f32r)
            outt_sb = wp.tile([128, NT, N], f32r)
            intt_v = intt_d.ap().rearrange("(t p) m -> p t m", p=128)
            outt_v = outt_d.ap().rearrange("(t p) m -> p t m", p=128)
            for t in range(NT):
                nc.gpsimd.dma_start(out=intt_sb[:, t, :], in_=intt_v[:, t, :])
                nc.gpsimd.dma_start(out=outt_sb[:, t, :], in_=outt_v[:, t, :])
            wts = {}
            for name, dr, ktiles in (("w3wt", w3wt_d, NT), ("w4wt", w4wt_d, NT),
                                     ("w5wt", w5wt_d, NT), ("w3ut", w3ut_d, DT),
                                     ("w5ut", w5ut_d, DT)):
                sb_t = wp.tile([128, ktiles, D], f32r, tag=name)
                v = dr.ap().rearrange("(t p) e -> p t e", p=128)
                for t in range(ktiles):
                    nc.gpsimd.dma_start(out=sb_t[:, t, :], in_=v[:, t, :])
                wts[name] = sb_t
            w3wt, w4wt, w5wt, w3ut, w5ut = (wts[k] for k in
                                            ("w3wt", "w4wt", "w5wt", "w3ut", "w5ut"))
            identr = wp.tile([128, 128], f32r)
            nc.gpsimd.dma_start(out=identr, in_=id_d.ap())
            biases = {}
            for name, dr in (("bz", bz_d), ("br", br_d), ("bh", bh_d)):
                bt = wp.tile([128, DT], f32, tag="bias_" + name)
                nc.sync.dma_start(out=bt, in_=dr.ap().rearrange("(t p) -> p t", p=128))
                biases[name] = bt
            bz_sb, br_sb, bh_sb = biases["bz"], biases["br"], biases["bh"]

            for b in range(BL):
                H = hp.tile([128, DT, N], f32r)
                xv = x_d.ap()[b].rearrange("(t p) n -> p t n", p=128)
                for t in range(DT):
                    nc.gpsimd.dma_start(out=H[:, t, :], in_=xv[:, t, :])

                for s in range(STEPS):
                    # T: transpose H (feature-major) -> Hnode (node-major)
                    Hn = hn.tile([128, NT, D], f32r)
                    for nt in range(NT):
                        for dt in range(DT):
                            pt = tp.tile([128, 128], f32r)
                            nc.tensor.transpose(
                                pt, H[:, dt, nt * 128:(nt + 1) * 128], identr)
                            nc.vector.tensor_copy(
                                Hn[:, nt, dt * 128:(dt + 1) * 128], pt)

                    for mc in range(MC):
                        ncs = slice(mc * CH, (mc + 1) * CH)
                        # G: av[ct] = adjacency.T-chunk contractions
                        av = avp.tile([128, 2 * DT, CH], f32r, tag="av")
                        for ai, adj in enumerate((intt_sb, outt_sb)):
                            for dt in range(DT):
                                pm = mm.tile([128, CH], f32)
                                for nt in range(NT):
                                    nc.tensor.matmul(
                                        pm,
                                        Hn[:, nt, dt * 128:(dt + 1) * 128],
                                        adj[:, nt, ncs],
                                        start=(nt == 0), stop=(nt == NT - 1))
                                nc.vector.tensor_copy(av[:, ai * DT + dt, :], pm)

                        # R: rv = sigmoid(w4w av + u3 + br); rvf = rv * f
                        rvf = rvfp.tile([128, DT, CH], f32r, tag="rvf")
                        for et in range(4):
                            ets = slice(et * 128, (et + 1) * 128)
                            pm = mm.tile([128, CH], f32)
                            for ct in range(2 * DT):
                                nc.tensor.matmul(pm, w4wt[:, ct, ets], av[:, ct, :],
                                                 start=(ct == 0), stop=False)
                            for dt in range(DT):
                                nc.tensor.matmul(pm, w3ut[:, dt, ets], H[:, dt, ncs],
                                                 start=False, stop=(dt == DT - 1))
                            rv = sc.tile([128, CH], f32, tag="sc")
                            nc.scalar.activation(rv, pm, AF.Sigmoid,
                                                 bias=br_sb[:, et:et + 1])
                            nc.vector.tensor_mul(rvf[:, et, :], rv, H[:, et, ncs])

                        # HV: hv = tanh(w5w av + w5u rvf + bh)
                        hv = hvp.tile([128, DT, CH], f32, tag="hv")
                        for et in range(4):
                            ets = slice(et * 128, (et + 1) * 128)
                            pm = mm.tile([128, CH], f32)
                            for ct in range(2 * DT):
                                nc.tensor.matmul(pm, w5wt[:, ct, ets], av[:, ct, :],
                                                 start=(ct == 0), stop=False)
                            for dt in range(DT):
                                nc.tensor.matmul(pm, w5ut[:, dt, ets], rvf[:, dt, :],
                                                 start=False, stop=(dt == DT - 1))
                            nc.scalar.activation(hv[:, et, :], pm, AF.Tanh,
                                                 bias=bh_sb[:, et:et + 1])

                        # Z: z = sigmoid(w3w av + u3 + bz); H += z*(hv-H)
                        zts = []
                        for et in range(4):
                            ets = slice(et * 128, (et + 1) * 128)
                            pm = mm.tile([128, CH], f32)
                            for ct in range(2 * DT):
                                nc.tensor.matmul(pm, w3wt[:, ct, ets], av[:, ct, :],
                                                 start=(ct == 0), stop=False)
                            for dt in range(DT):
                                nc.tensor.matmul(pm, w3ut[:, dt, ets], H[:, dt, ncs],
                                                 start=False, stop=(dt == DT - 1))
                            z = ztp.tile([128, CH], f32, tag="zt")
                            nc.scalar.activation(z, pm, AF.Sigmoid,
                                                 bias=bz_sb[:, et:et + 1])
                            zts.append(z)
                        for et in range(4):
                            t2 = sc.tile([128, CH], f32, tag="sc")
                            nc.vector.tensor_sub(t2, hv[:, et, :], H[:, et, ncs])
                            nc.vector.tensor_mul(t2, zts[et], t2)
                            nc.vector.tensor_add(H[:, et, ncs], H[:, et, ncs], t2)

                yv = y_d.ap()[b].rearrange("(t p) n -> p t n", p=128)
                for t in range(DT):
                    nc.gpsimd.dma_start(out=yv[:, t, :], in_=H[:, t, :])

    _install_wait_splitter(nc)
    return nc


_NC = None


def _get_nc():
    global _NC
    if _NC is None:
        _NC = build_nc()
    return _NC


def kernel(x, in_matrix, out_matrix, w3w, b3w, w3u, b3u, w4w, b4w,
           w5w, b5w, w5u, b5u, **trace_kwargs):
    x = np.ascontiguousarray(np.asarray(x, dtype=np.float32))
    shared = {
        "intt": np.ascontiguousarray(np.asarray(in_matrix).T),
        "outt": np.ascontiguousarray(np.asarray(out_matrix).T),
        "w3wt": np.ascontiguousarray(np.asarray(w3w).T),
        "w4wt": np.ascontiguousarray(np.asarray(w4w).T),
        "w5wt": np.ascontiguousarray(np.asarray(w5w).T),
        "w3ut": np.ascontiguousarray(np.asarray(w3u).T),
        "w5ut": np.ascontiguousarray(np.asarray(w5u).T),
        "bz": np.asarray(b3w) + np.asarray(b3u),
        "br": np.asarray(b4w) + np.asarray(b3u),
        "bh": np.asarray(b5w) + np.asarray(b5u),
        "ident": np.eye(128, dtype=np.float32),
    }
    xf = np.ascontiguousarray(x.transpose(0, 2, 1))  # [B, D, N]
    in_maps = []
    for c in range(NCORES):
        m = dict(shared)
        m["x"] = np.ascontiguousarray(xf[c * BL:(c + 1) * BL])
        in_maps.append(m)
    res = run_bass_kernel_spmd(_get_nc(), in_maps,
                               core_ids=list(range(NCORES)), **trace_kwargs)
    y = np.concatenate([r["y"] for r in res.results], axis=0)  # [B, D, N]
    nodes = np.ascontiguousarray(y.transpose(0, 2, 1))         # [B, N, D]
    kernel.last_results = res
    return nodes, np.asarray(in_matrix)
